# revision 1
# baseline (speedup 1.0000x reference)
"""Bipolar self-attention on 8 Trainium2 NeuronCores.

Sharding: data-parallel over batch (B=2 -> 2 groups of 4 cores), tensor-
parallel over heads within a group (16 heads -> 4 heads/core). Each core:
  - projects its head-slice of Q/K transposed ([c, n] layout) and V natural,
    with the bipolar transform (q-0.5)*2 and the 1/sqrt(Dh) score scale
    folded into the projection weights/biases host-side,
  - computes S^T = Kb Qb^T per head tile-by-tile, exponentiates (softmax
    without max subtraction -- scores are O(10), exp is safe in fp32),
  - multiplies P^T by V augmented with a ones column, so the softmax
    denominator falls out of the same matmul (row 64 of the accumulator),
  - normalizes and applies its slice of the output projection (row-parallel).
Host sums the 4 partial outputs per batch and adds the bias terms.

All matmuls run in float32r (full fp32 storage, reduced-precision multiply)
which is 4x faster than fp32 on the PE at moving-dim >= 256. Head pairs are
packed into disjoint PE row groups (partitions 0-63 / 64-127) so the K=64
QK^T matmuls of two heads run concurrently, and one ACTIVATE exponentiates
both heads' scores.
"""

import numpy as np

import concourse.bass as bass
import concourse.tile as tile
from concourse import bacc, mybir
from concourse.bass_utils import run_bass_kernel_spmd

D_MODEL = 1024
NHEAD = 16
HEAD_DIM = 64
B = 2
N = 2048
N_CORES = 8
HEADS_PER_CORE = NHEAD // (N_CORES // B)  # 4
C_LOC = HEADS_PER_CORE * HEAD_DIM  # 256

F32 = mybir.dt.float32
F32R = mybir.dt.float32r

_CACHE = {}


def build_nc():
    nc = bacc.Bacc("TRN2", target_bir_lowering=False, debug=False)

    xT = nc.dram_tensor("xT", [D_MODEL, N], F32R, kind="ExternalInput")
    wqT = nc.dram_tensor("wqT", [D_MODEL, C_LOC], F32R, kind="ExternalInput")
    wkT = nc.dram_tensor("wkT", [D_MODEL, C_LOC], F32R, kind="ExternalInput")
    wvT = nc.dram_tensor("wvT", [D_MODEL, C_LOC], F32R, kind="ExternalInput")
    woT = nc.dram_tensor("woT", [C_LOC, D_MODEL], F32R, kind="ExternalInput")
    bq = nc.dram_tensor("bq", [C_LOC], F32, kind="ExternalInput")
    bk = nc.dram_tensor("bk", [C_LOC], F32, kind="ExternalInput")
    y = nc.dram_tensor("y", [N, D_MODEL], F32, kind="ExternalOutput")

    NT = N // 128          # 16 k tiles
    DC = D_MODEL // 128    # 8 contraction chunks
    CT = C_LOC // 128      # 2 local-channel tiles
    QW = 512               # q window width

    with tile.TileContext(nc) as tc:
        with (
            tc.tile_pool(name="singles", bufs=1) as singles,
            tc.tile_pool(name="pt", bufs=3) as ptp,
            tc.tile_pool(name="ovs", bufs=4) as ovsp,
            tc.tile_pool(name="norm", bufs=3) as normp,
            tc.tile_pool(name="yout", bufs=2) as youtp,
        ):
            # weights/biases first so the first projection matmuls start early
            bq_sb = singles.tile([128, CT], F32)
            nc.sync.dma_start(bq_sb[:], bq.ap().rearrange("(c p) -> p c", p=128))
            bk_sb = singles.tile([128, CT], F32)
            nc.sync.dma_start(bk_sb[:], bk.ap().rearrange("(c p) -> p c", p=128))
            wqT_sb = singles.tile([128, DC, C_LOC], F32R)
            nc.sync.dma_start(wqT_sb[:], wqT.ap().rearrange("(c p) m -> p c m", p=128))
            wkT_sb = singles.tile([128, DC, C_LOC], F32R)
            nc.sync.dma_start(wkT_sb[:], wkT.ap().rearrange("(c p) m -> p c m", p=128))
            wvT_sb = singles.tile([128, DC, C_LOC], F32R)
            nc.sync.dma_start(wvT_sb[:], wvT.ap().rearrange("(c p) m -> p c m", p=128))
            woT_sb = singles.tile([128, CT, D_MODEL], F32R)
            nc.sync.dma_start(woT_sb[:], woT.ap().rearrange("(c p) m -> p c m", p=128))
            xT_sb = singles.tile([128, DC, N], F32R)
            xT_r = xT.ap().rearrange("(c p) n -> p c n", p=128)
            for dc in range(DC):
                nc.sync.dma_start(xT_sb[:, dc], xT_r[:, dc])

            qT_sb = singles.tile([128, CT, N], F32R)
            kT_sb = singles.tile([128, CT, N], F32R)
            # V per k-tile per head, with a trailing ones column (col 64)
            v1_sb = singles.tile([128, NT, HEADS_PER_CORE, HEAD_DIM + 1], F32R)
            ones_sb = singles.tile([128, NT * HEADS_PER_CORE], F32)
            nc.vector.memset(ones_sb[:], 1.0)
            nc.vector.tensor_copy(
                v1_sb[:, :, :, HEAD_DIM],
                ones_sb[:].rearrange("p (n h) -> p n h", h=HEADS_PER_CORE),
            )
            outT_sb = singles.tile([128, CT, N], F32R)
            onescol = singles.tile([1, 64], F32R)
            nc.vector.tensor_copy(onescol[:], ones_sb[0:1, 0:64])

            # ---- projections.  qT[c, n] = sum_d wqT[d, c] xT[d, n] (+bias);
            # order: q/k of c-tile 0, V, q/k of c-tile 1 -- so head pair 0 can
            # start attention while the rest projects.
            def qk_proj(w_sb, b_sb, dst, ct, pps):
                for nch in range(N // 512):
                    ps = pps.tile([128, 512], F32, tag="qk")
                    for dc in range(DC):
                        nc.tensor.matmul(
                            ps[:],
                            w_sb[:, dc, ct * 128:(ct + 1) * 128],
                            xT_sb[:, dc, nch * 512:(nch + 1) * 512],
                            start=(dc == 0),
                            stop=(dc == DC - 1),
                        )
                    nc.vector.tensor_tensor(
                        dst[:, ct, nch * 512:(nch + 1) * 512],
                        ps[:],
                        b_sb[:, ct:ct + 1].to_broadcast((128, 512)),
                        mybir.AluOpType.add,
                    )


            def make_v_proj_tile(pps):
                # V natural: v[n, c] = sum_d xT[d, n] wvT[d, c], one n tile
                def v_proj_tile(nt):
                    ps = pps.tile([128, 512], F32, tag="qk")
                    for dc in range(DC):
                        nc.tensor.matmul(
                            ps[:, :C_LOC],
                            xT_sb[:, dc, nt * 128:(nt + 1) * 128],
                            wvT_sb[:, dc, :],
                            start=(dc == 0),
                            stop=(dc == DC - 1),
                        )
                    nc.vector.tensor_copy(
                        v1_sb[:, nt, :, 0:HEAD_DIM],
                        ps[:, :C_LOC].rearrange("p (h d) -> p h d",
                                                h=HEADS_PER_CORE),
                    )
                return v_proj_tile

            with (
                tc.tile_pool(name="proj_ps", bufs=2, space="PSUM") as pps,
                tc.tile_pool(name="st_ps", bufs=2, space="PSUM") as stp,
                tc.tile_pool(name="ovy_ps", bufs=2, space="PSUM") as ovp,
                tc.tile_pool(name="dsc", bufs=4, space="DRAM") as dscp,
            ):
                def attention_pair(qq, pair, vproj_interleave=False):
                    q0 = qq * QW
                    ct_h = pair
                    ovA = ovp.tile([HEAD_DIM + 1, QW], F32, tag="ov")
                    ovB = ovp.tile([HEAD_DIM + 1, QW], F32, tag="ov")
                    for kt in range(NT):
                        if vproj_interleave:
                            v_proj_tile(kt)
                        st = stp.tile([128, 2 * QW], F32)
                        for half, p0 in ((0, 0), (1, 64)):
                            nc.tensor.matmul(
                                st[:, half * QW:(half + 1) * QW],
                                kT_sb[p0:p0 + 64, ct_h,
                                      kt * 128:(kt + 1) * 128],
                                qT_sb[p0:p0 + 64, ct_h, q0:q0 + QW],
                                start=True,
                                stop=True,
                            )
                        pt = ptp.tile([128, 2 * QW], F32R)
                        nc.scalar.activation(
                            pt[:], st[:], mybir.ActivationFunctionType.Exp
                        )
                        for half, ov in ((0, ovA), (1, ovB)):
                            nc.tensor.matmul(
                                ov[:],
                                v1_sb[:, kt, 2 * pair + half, :],
                                pt[:, half * QW:(half + 1) * QW],
                                start=(kt == 0),
                                stop=(kt == NT - 1),
                            )
                    # copy accumulators out of PSUM fast, then normalize
                    for half, ov in ((0, ovA), (1, ovB)):
                        p0 = 64 * half
                        ovs = ovsp.tile([HEAD_DIM + 1, QW], F32, tag="ovs")
                        nc.vector.tensor_copy(ovs[:], ov[:])
                        rec = normp.tile([1, QW], F32, tag="nrm")
                        nc.vector.reciprocal(
                            rec[:], ovs[HEAD_DIM:HEAD_DIM + 1, :]
                        )
                        rdram = dscp.tile([1, QW], F32)
                        nc.sync.dma_start(rdram[:], rec[:])
                        bc = normp.tile([64, QW], F32, tag="nrm")
                        nc.gpsimd.dma_start(
                            bc[:], rdram[:].partition_broadcast(64)
                        )
                        nc.vector.tensor_mul(
                            outT_sb[p0:p0 + 64, ct_h, q0:q0 + QW],
                            ovs[0:HEAD_DIM, :],
                            bc[:],
                        )

                def y_proj(qq):
                    # output projection for one finished q window
                    for nt in range(qq * QW // 128, (qq + 1) * QW // 128):
                        for cok in range(D_MODEL // 512):
                            ps = ovp.tile([128, 512], F32, tag="ov")
                            for ct in range(CT):
                                nc.tensor.matmul(
                                    ps[:],
                                    outT_sb[:, ct, nt * 128:(nt + 1) * 128],
                                    woT_sb[:, ct, cok * 512:(cok + 1) * 512],
                                    start=(ct == 0),
                                    stop=(ct == CT - 1),
                                )
                            ys = youtp.tile([128, 512], F32, tag="ys")
                            nc.vector.tensor_copy(ys[:], ps[:])
                            nc.sync.dma_start(
                                y.ap()[nt * 128:(nt + 1) * 128,
                                       cok * 512:(cok + 1) * 512],
                                ys[:],
                            )

                # interleaved emission: attention starts as soon as c-tile 0
                # of q/k is projected; y projection trails one window so its
                # matmuls never head-of-line block the PE stream.
                v_proj_tile = make_v_proj_tile(pps)
                qk_proj(wqT_sb, bq_sb, qT_sb, 0, pps)
                qk_proj(wkT_sb, bk_sb, kT_sb, 0, pps)
                for _nt in range(NT):
                    v_proj_tile(_nt)
                attention_pair(0, 0)
                qk_proj(wqT_sb, bq_sb, qT_sb, 1, pps)
                qk_proj(wkT_sb, bk_sb, kT_sb, 1, pps)
                attention_pair(0, 1)
                for qq in range(1, N // QW):
                    attention_pair(qq, 0)
                    attention_pair(qq, 1)
                    y_proj(qq - 1)
                y_proj(N // QW - 1)

    nc.compile()
    return nc


def kernel(x, Wq, bq, Wk, bk, Wv, bv, Wo, bo):
    x = np.asarray(x, dtype=np.float32)
    Wq = np.asarray(Wq, dtype=np.float32)
    Wk = np.asarray(Wk, dtype=np.float32)
    Wv = np.asarray(Wv, dtype=np.float32)
    Wo = np.asarray(Wo, dtype=np.float32)
    bq = np.asarray(bq, dtype=np.float32)
    bk = np.asarray(bk, dtype=np.float32)
    bv = np.asarray(bv, dtype=np.float32)
    bo = np.asarray(bo, dtype=np.float32)

    if "nc" not in _CACHE:
        _CACHE["nc"] = build_nc()
    nc = _CACHE["nc"]

    s = 2.0 / np.sqrt(8.0)  # fold bipolar *2 and score scale (1/8 split per side)
    in_maps = []
    for core in range(N_CORES):
        b = core // (N_CORES // B)
        g = core % (N_CORES // B)
        ch = slice(g * C_LOC, (g + 1) * C_LOC)
        in_maps.append({
            "xT": np.ascontiguousarray(x[b].T),
            "wqT": np.ascontiguousarray((s * Wq[ch, :]).T),
            "wkT": np.ascontiguousarray((s * Wk[ch, :]).T),
            "wvT": np.ascontiguousarray(Wv[ch, :].T),
            "woT": np.ascontiguousarray(Wo[:, ch].T),
            "bq": ((2.0 * bq[ch] - 1.0) / np.sqrt(8.0)).astype(np.float32),
            "bk": ((2.0 * bk[ch] - 1.0) / np.sqrt(8.0)).astype(np.float32),
        })

    _CACHE["in_maps"] = in_maps
    res = run_bass_kernel_spmd(nc, in_maps, core_ids=list(range(N_CORES)))

    g_per_b = N_CORES // B
    const = (Wo @ bv + bo).astype(np.float32)  # bv folded through out-proj
    out = np.empty((B, N, D_MODEL), dtype=np.float32)
    for b in range(B):
        acc = res.results[b * g_per_b]["y"].astype(np.float32).copy()
        for g in range(1, g_per_b):
            acc += res.results[b * g_per_b + g]["y"]
        out[b] = acc + const
    return out



# revision 9
# speedup vs baseline: 1.0073x; 1.0073x over previous
"""Bipolar self-attention on 8 Trainium2 NeuronCores.

Sharding: data-parallel over batch (B=2 -> 2 groups of 4 cores), tensor-
parallel over heads within a group (16 heads -> 4 heads/core). Each core:
  - projects its head-slice of Q/K transposed ([c, n] layout) and V natural,
    with the bipolar transform (q-0.5)*2 and the 1/sqrt(Dh) score scale
    folded into the projection weights/biases host-side,
  - computes S^T = Kb Qb^T per head tile-by-tile, exponentiates (softmax
    without max subtraction -- scores are O(10), exp is safe in fp32),
  - multiplies P^T by a [V_A | ones | V_B] stationary block: the PV matmul
    for head A uses cols 0-127 ([V_A | ones]) so PSUM rows 0-63 hold the
    attention output and rows 64-127 hold the softmax denominator already
    replicated across 64 partitions; head B uses cols 64-191 ([ones | V_B])
    with the roles of the row halves flipped.  Matmul cost depends only on
    the moving dim, so the denominator broadcast is free,
  - normalizes with one reciprocal_approx_fast + one tensor_mul straight
    from PSUM (no DRAM broadcast roundtrip, no PSUM evacuation copy),
  - applies its slice of the output projection (row-parallel).
Host sums the 4 partial outputs per batch and adds the bias terms.

All matmuls run in float32r (1 cycle/row at moving>=256).  The PE executes
in order, so independent projection / output-projection matmuls are
interleaved INTO the attention k-tile loops to fill the PE's exp-wait gaps,
and emission is pair-major (all 4 q-windows of head-pair 0, then of pair 1)
so the second pair's Q/K projection spreads across pair 0's ACT-bound slack.
"""

import numpy as np

import concourse.bass as bass
import concourse.tile as tile
from concourse import bacc, mybir
from concourse.bass_utils import run_bass_kernel_spmd

D_MODEL = 1024
NHEAD = 16
HEAD_DIM = 64
B = 2
N = 2048
N_CORES = 8
HEADS_PER_CORE = NHEAD // (N_CORES // B)  # 4
C_LOC = HEADS_PER_CORE * HEAD_DIM  # 256

F32 = mybir.dt.float32
F32R = mybir.dt.float32r

_CACHE = {}


def build_nc():
    nc = bacc.Bacc("TRN2", target_bir_lowering=False, debug=False)

    xT = nc.dram_tensor("xT", [D_MODEL, N], F32R, kind="ExternalInput")
    wqT = nc.dram_tensor("wqT", [D_MODEL, C_LOC], F32R, kind="ExternalInput")
    wkT = nc.dram_tensor("wkT", [D_MODEL, C_LOC], F32R, kind="ExternalInput")
    wvT = nc.dram_tensor("wvT", [D_MODEL, C_LOC], F32R, kind="ExternalInput")
    woT = nc.dram_tensor("woT", [C_LOC, D_MODEL], F32R, kind="ExternalInput")
    bq = nc.dram_tensor("bq", [C_LOC], F32, kind="ExternalInput")
    bk = nc.dram_tensor("bk", [C_LOC], F32, kind="ExternalInput")
    y = nc.dram_tensor("y", [N, D_MODEL], F32, kind="ExternalOutput")

    NT = N // 128          # 16 k tiles
    DC = D_MODEL // 128    # 8 contraction chunks
    CT = C_LOC // 128      # 2 local-channel tiles (= head pairs)
    QW = 512               # q window width
    NW = N // QW           # 4 q windows

    with tile.TileContext(nc) as tc:
        with (
            tc.tile_pool(name="singles", bufs=1) as singles,
            tc.tile_pool(name="pt", bufs=3) as ptp,
            tc.tile_pool(name="rec", bufs=4) as recp,
            tc.tile_pool(name="yout", bufs=2) as youtp,
        ):
            # small biases first, then the weights/x slices the first
            # projection chain needs, so the PE can start ~6us in.
            bq_sb = singles.tile([128, CT], F32)
            nc.sync.dma_start(bq_sb[:], bq.ap().rearrange("(c p) -> p c", p=128))
            bk_sb = singles.tile([128, CT], F32)
            nc.sync.dma_start(bk_sb[:], bk.ap().rearrange("(c p) -> p c", p=128))
            wqT_sb = singles.tile([128, DC, C_LOC], F32R)
            nc.sync.dma_start(wqT_sb[:], wqT.ap().rearrange("(c p) m -> p c m", p=128))
            xT_sb = singles.tile([128, DC, N], F32R)
            xT_r = xT.ap().rearrange("(c p) n -> p c n", p=128)
            for dc in range(DC):
                nc.sync.dma_start(xT_sb[:, dc, 0:QW], xT_r[:, dc, 0:QW])
            wkT_sb = singles.tile([128, DC, C_LOC], F32R)
            nc.sync.dma_start(wkT_sb[:], wkT.ap().rearrange("(c p) m -> p c m", p=128))
            wvT_sb = singles.tile([128, DC, C_LOC], F32R)
            nc.sync.dma_start(wvT_sb[:], wvT.ap().rearrange("(c p) m -> p c m", p=128))
            for blk in range(1, NW):
                for dc in range(DC):
                    nc.sync.dma_start(
                        xT_sb[:, dc, blk * QW:(blk + 1) * QW],
                        xT_r[:, dc, blk * QW:(blk + 1) * QW],
                    )
            woT_sb = singles.tile([128, CT, D_MODEL], F32R)
            nc.sync.dma_start(woT_sb[:], woT.ap().rearrange("(c p) m -> p c m", p=128))

            qT_sb = singles.tile([128, CT, N], F32R)
            kT_sb = singles.tile([128, CT, N], F32R)
            # V stationary blocks: per (k-tile, pair, half) a [128, 128]
            # block [V_head (64) | ones (64)]: PV output rows 0:64 are the
            # attention output, rows 64:128 the softmax denominator
            # replicated across partitions (broadcast for free).
            v1_sb = singles.tile([128, NT, CT, 2, 128], F32R)
            ones_sb = singles.tile([128, 128], F32)
            nc.vector.memset(ones_sb[:], 1.0)
            for nt in range(NT):
                for pair in range(CT):
                    nc.vector.tensor_copy(
                        v1_sb[:, nt, pair, :, 64:128],
                        ones_sb[:].rearrange("p (h d) -> p h d", h=2),
                    )
            outT_sb = singles.tile([128, CT, N], F32R)

            # ---- emission helpers.  All PE work is emitted via closures so
            # the interleaving below is explicit.
            with (
                tc.tile_pool(name="ps512", bufs=2, space="PSUM") as psp,
                tc.tile_pool(name="st_ps", bufs=2, space="PSUM") as stp,
                tc.tile_pool(name="ov_ps", bufs=2, space="PSUM") as ovp,
            ):
                def qk_proj_chunk(w_sb, b_sb, dst, ct, nch):
                    # one 512-wide chunk: 8 matmuls + bias add
                    ps = psp.tile([128, 512], F32, tag="ps")
                    for dc in range(DC):
                        nc.tensor.matmul(
                            ps[:],
                            w_sb[:, dc, ct * 128:(ct + 1) * 128],
                            xT_sb[:, dc, nch * 512:(nch + 1) * 512],
                            start=(dc == 0),
                            stop=(dc == DC - 1),
                        )
                    nc.vector.tensor_tensor(
                        dst[:, ct, nch * 512:(nch + 1) * 512],
                        ps[:],
                        b_sb[:, ct:ct + 1].to_broadcast((128, 512)),
                        mybir.AluOpType.add,
                    )

                def v_proj_tile(nt):
                    # V natural: v[n, c] = sum_d xT[d, n] wvT[d, c], one n tile
                    ps = psp.tile([128, 512], F32, tag="ps")
                    for dc in range(DC):
                        nc.tensor.matmul(
                            ps[:, :C_LOC],
                            xT_sb[:, dc, nt * 128:(nt + 1) * 128],
                            wvT_sb[:, dc, :],
                            start=(dc == 0),
                            stop=(dc == DC - 1),
                        )
                    # scatter the 4 heads into their [pair, half] slots
                    nc.vector.tensor_copy(
                        v1_sb[:, nt].rearrange("p c h w -> p (c h) w")[:, :, 0:64],
                        ps[:, :C_LOC].rearrange("p (g d) -> p g d", g=4),
                    )

                def y_proj_chunk(nt, cok):
                    # output projection for one (row-tile, 512-col) chunk
                    ps = psp.tile([128, 512], F32, tag="ps")
                    for ct in range(CT):
                        nc.tensor.matmul(
                            ps[:],
                            outT_sb[:, ct, nt * 128:(nt + 1) * 128],
                            woT_sb[:, ct, cok * 512:(cok + 1) * 512],
                            start=(ct == 0),
                            stop=(ct == CT - 1),
                        )
                    ys = youtp.tile([128, 512], F32, tag="ys")
                    nc.vector.tensor_copy(ys[:], ps[:])
                    nc.sync.dma_start(
                        y.ap()[nt * 128:(nt + 1) * 128,
                               cok * 512:(cok + 1) * 512],
                        ys[:],
                    )

                def attention_window(qq, pair, filler=None):
                    """One q-window of one head pair.  `filler()` is called
                    once per k-tile to emit interleaved PE work."""
                    q0 = qq * QW
                    ovA = ovp.tile([128, QW], F32, tag="ov")
                    ovB = ovp.tile([128, QW], F32, tag="ov")
                    for kt in range(NT):
                        st = stp.tile([128, 2 * QW], F32)
                        for half, p0 in ((0, 0), (1, 64)):
                            nc.tensor.matmul(
                                st[:, half * QW:(half + 1) * QW],
                                kT_sb[p0:p0 + 64, pair,
                                      kt * 128:(kt + 1) * 128],
                                qT_sb[p0:p0 + 64, pair, q0:q0 + QW],
                                start=True,
                                stop=True,
                            )
                        pt = ptp.tile([128, 2 * QW], F32R)
                        nc.scalar.activation(
                            pt[:], st[:], mybir.ActivationFunctionType.Exp
                        )
                        if filler is not None:
                            filler(kt)
                        # [V | ones] -> out rows 0:64, den rows 64:128
                        for half, ov in ((0, ovA), (1, ovB)):
                            nc.tensor.matmul(
                                ov[:],
                                v1_sb[:, kt, pair, half],
                                pt[:, half * QW:(half + 1) * QW],
                                start=(kt == 0),
                                stop=(kt == NT - 1),
                            )
                    # normalize: rec = 1/den (broadcast across partitions is
                    # already materialized), outT = out * rec
                    for half, ov in ((0, ovA), (1, ovB)):
                        p0 = 64 * half
                        rec = recp.tile([64, QW], F32, tag="rec")
                        nc.vector.reciprocal(rec[:], ov[64:128, :])
                        nc.vector.tensor_mul(
                            outT_sb[p0:p0 + 64, pair, q0:q0 + QW],
                            ov[0:64, :],
                            rec[:],
                        )

                # ---- schedule -------------------------------------------
                # Q/K projections for pair 0, then V for the first tiles so
                # attention can start; V for k-tile kt+2 is emitted inside
                # window (0,0)'s loop.
                for nch in range(NW):
                    qk_proj_chunk(wqT_sb, bq_sb, qT_sb, 0, nch)
                    qk_proj_chunk(wkT_sb, bk_sb, kT_sb, 0, nch)
                v_proj_tile(0)
                v_proj_tile(1)

                def fill_v(kt):
                    if kt + 2 < NT:
                        v_proj_tile(kt + 2)

                attention_window(0, 0, filler=fill_v)

                # pair-0 windows 1..3; spread pair-1 Q/K projection (8
                # chunks) across the first two windows' slack.
                proj1 = []
                for nch in range(NW):
                    proj1.append((wqT_sb, bq_sb, qT_sb, nch))
                    proj1.append((wkT_sb, bk_sb, kT_sb, nch))

                def make_fill_proj(chunks):
                    it = iter(chunks)

                    def fill(kt):
                        if kt % 4 == 1:
                            args = next(it, None)
                            if args is not None:
                                w, b, dst, nch = args
                                qk_proj_chunk(w, b, dst, 1, nch)
                    return fill

                fp = make_fill_proj(proj1)
                attention_window(1, 0, filler=fp)
                attention_window(2, 0, filler=fp)
                attention_window(3, 0, filler=fp)

                # pair-1 windows with trailing y projection interleaved.
                # y window qq is ready once pair-1 window qq is normalized.
                def make_fill_y(qq_ready):
                    chunks = [(nt, cok)
                              for nt in range(qq_ready * (QW // 128),
                                              (qq_ready + 1) * (QW // 128))
                              for cok in range(D_MODEL // 512)]
                    it = iter(chunks)

                    def fill(kt):
                        if kt % 2 == 0:
                            args = next(it, None)
                            if args is not None:
                                y_proj_chunk(*args)
                    return fill

                attention_window(0, 1)
                attention_window(1, 1, filler=make_fill_y(0))
                attention_window(2, 1, filler=make_fill_y(1))
                attention_window(3, 1, filler=make_fill_y(2))
                for nt in range(3 * (QW // 128), NW * (QW // 128)):
                    for cok in range(D_MODEL // 512):
                        y_proj_chunk(nt, cok)

    nc.compile()
    return nc


def kernel(x, Wq, bq, Wk, bk, Wv, bv, Wo, bo):
    x = np.asarray(x, dtype=np.float32)
    Wq = np.asarray(Wq, dtype=np.float32)
    Wk = np.asarray(Wk, dtype=np.float32)
    Wv = np.asarray(Wv, dtype=np.float32)
    Wo = np.asarray(Wo, dtype=np.float32)
    bq = np.asarray(bq, dtype=np.float32)
    bk = np.asarray(bk, dtype=np.float32)
    bv = np.asarray(bv, dtype=np.float32)
    bo = np.asarray(bo, dtype=np.float32)

    if "nc" not in _CACHE:
        _CACHE["nc"] = build_nc()
    nc = _CACHE["nc"]

    s = 2.0 / np.sqrt(8.0)  # fold bipolar *2 and score scale (1/8 split per side)
    in_maps = []
    for core in range(N_CORES):
        b = core // (N_CORES // B)
        g = core % (N_CORES // B)
        ch = slice(g * C_LOC, (g + 1) * C_LOC)
        in_maps.append({
            "xT": np.ascontiguousarray(x[b].T),
            "wqT": np.ascontiguousarray((s * Wq[ch, :]).T),
            "wkT": np.ascontiguousarray((s * Wk[ch, :]).T),
            "wvT": np.ascontiguousarray(Wv[ch, :].T),
            "woT": np.ascontiguousarray(Wo[:, ch].T),
            "bq": ((2.0 * bq[ch] - 1.0) / np.sqrt(8.0)).astype(np.float32),
            "bk": ((2.0 * bk[ch] - 1.0) / np.sqrt(8.0)).astype(np.float32),
        })

    _CACHE["in_maps"] = in_maps
    res = run_bass_kernel_spmd(nc, in_maps, core_ids=list(range(N_CORES)))

    g_per_b = N_CORES // B
    const = (Wo @ bv + bo).astype(np.float32)  # bv folded through out-proj
    out = np.empty((B, N, D_MODEL), dtype=np.float32)
    for b in range(B):
        acc = res.results[b * g_per_b]["y"].astype(np.float32).copy()
        for g in range(1, g_per_b):
            acc += res.results[b * g_per_b + g]["y"]
        out[b] = acc + const
    return out


# revision 17
# speedup vs baseline: 1.2390x; 1.2300x over previous
"""Bipolar self-attention on 8 Trainium2 NeuronCores.

Sharding: data-parallel over batch (B=2 -> 2 groups of 4 cores), tensor-
parallel over heads within a group (16 heads -> 4 heads/core). Each core:
  - projects its head-slice of Q/K transposed ([c, n] layout) and V natural,
    with the bipolar transform (q-0.5)*2 and the 1/sqrt(Dh) score scale
    folded into the projection weights/biases host-side,
  - computes S^T = Kb Qb^T per head tile-by-tile, exponentiates (softmax
    without max subtraction -- scores are O(10), exp is safe in fp32),
  - multiplies P^T by a [V_A | ones | V_B] stationary block: the PV matmul
    for head A uses cols 0-127 ([V_A | ones]) so PSUM rows 0-63 hold the
    attention output and rows 64-127 hold the softmax denominator already
    replicated across 64 partitions; head B uses cols 64-191 ([ones | V_B])
    with the roles of the row halves flipped.  Matmul cost depends only on
    the moving dim, so the denominator broadcast is free,
  - normalizes with one reciprocal_approx_fast + one tensor_mul straight
    from PSUM (no DRAM broadcast roundtrip, no PSUM evacuation copy),
  - applies its slice of the output projection (row-parallel).
Host sums the 4 partial outputs per batch and adds the bias terms.

All matmuls run in float32r (1 cycle/row at moving>=256).  The PE executes
in order, so independent projection / output-projection matmuls are
interleaved INTO the attention k-tile loops to fill the PE's exp-wait gaps,
and emission is pair-major (all 4 q-windows of head-pair 0, then of pair 1)
so the second pair's Q/K projection spreads across pair 0's ACT-bound slack.
"""

import ml_dtypes
import numpy as np

import concourse.bass as bass
import concourse.tile as tile
from concourse import bacc, mybir
from concourse.bass_utils import run_bass_kernel_spmd

D_MODEL = 1024
NHEAD = 16
HEAD_DIM = 64
B = 2
N = 2048
N_CORES = 8
HEADS_PER_CORE = NHEAD // (N_CORES // B)  # 4
C_LOC = HEADS_PER_CORE * HEAD_DIM  # 256

F32 = mybir.dt.float32
F32R = mybir.dt.float32r
BF16 = mybir.dt.bfloat16

_CACHE = {}


def build_nc():
    nc = bacc.Bacc("TRN2", target_bir_lowering=False, debug=False)

    xT = nc.dram_tensor("xT", [D_MODEL, N], F32R, kind="ExternalInput")
    wqT = nc.dram_tensor("wqT", [D_MODEL, C_LOC], F32R, kind="ExternalInput")
    wkT = nc.dram_tensor("wkT", [D_MODEL, C_LOC], F32R, kind="ExternalInput")
    wvT = nc.dram_tensor("wvT", [D_MODEL, C_LOC], F32R, kind="ExternalInput")
    woT = nc.dram_tensor("woT", [C_LOC, D_MODEL], BF16, kind="ExternalInput")
    bq = nc.dram_tensor("bq", [C_LOC], F32, kind="ExternalInput")
    bk = nc.dram_tensor("bk", [C_LOC], F32, kind="ExternalInput")
    y = nc.dram_tensor("y", [N, D_MODEL], F32, kind="ExternalOutput")

    NT = N // 128          # 16 k tiles
    DC = D_MODEL // 128    # 8 contraction chunks
    CT = C_LOC // 128      # 2 local-channel tiles (= head pairs)
    QW = 512               # q window width
    NW = N // QW           # 4 q windows

    with tile.TileContext(nc) as tc:
        with (
            tc.tile_pool(name="singles", bufs=1) as singles,
            tc.tile_pool(name="pt", bufs=3) as ptp,
            tc.tile_pool(name="rec", bufs=4) as recp,
            tc.tile_pool(name="yout", bufs=2) as youtp,
        ):
            # small biases first, then the weights/x slices the first
            # projection chain needs, so the PE can start ~6us in.
            bq_sb = singles.tile([128, CT], F32)
            nc.sync.dma_start(bq_sb[:], bq.ap().rearrange("(c p) -> p c", p=128))
            bk_sb = singles.tile([128, CT], F32)
            nc.sync.dma_start(bk_sb[:], bk.ap().rearrange("(c p) -> p c", p=128))
            wqT_sb = singles.tile([128, DC, C_LOC], F32R)
            nc.sync.dma_start(wqT_sb[:], wqT.ap().rearrange("(c p) m -> p c m", p=128))
            xT_sb = singles.tile([128, DC, N], F32R)
            xT_r = xT.ap().rearrange("(c p) n -> p c n", p=128)
            for dc in range(DC):
                nc.sync.dma_start(xT_sb[:, dc, 0:QW], xT_r[:, dc, 0:QW])
            wkT_sb = singles.tile([128, DC, C_LOC], F32R)
            nc.sync.dma_start(wkT_sb[:], wkT.ap().rearrange("(c p) m -> p c m", p=128))
            wvT_sb = singles.tile([128, DC, C_LOC], F32R)
            nc.sync.dma_start(wvT_sb[:], wvT.ap().rearrange("(c p) m -> p c m", p=128))
            for blk in range(1, NW):
                for dc in range(DC):
                    nc.sync.dma_start(
                        xT_sb[:, dc, blk * QW:(blk + 1) * QW],
                        xT_r[:, dc, blk * QW:(blk + 1) * QW],
                    )
            woT_sb = singles.tile([128, CT, D_MODEL], BF16)
            nc.sync.dma_start(woT_sb[:], woT.ap().rearrange("(c p) m -> p c m", p=128))

            qT_sb = singles.tile([128, CT, N], F32R)
            kT_sb = singles.tile([128, CT, N], F32R)
            # V stationary blocks: per (k-tile, pair, half) a [128, 128]
            # block [V_head (64) | ones (64)]: PV output rows 0:64 are the
            # attention output, rows 64:128 the softmax denominator
            # replicated across partitions (broadcast for free).
            v1_sb = singles.tile([128, NT, CT, 2, 128], BF16)
            ones_sb = singles.tile([128, 128], F32)
            nc.vector.memset(ones_sb[:], 1.0)
            for nt in range(NT):
                for pair in range(CT):
                    nc.vector.tensor_copy(
                        v1_sb[:, nt, pair, :, 64:128],
                        ones_sb[:].rearrange("p (h d) -> p h d", h=2),
                    )
            outT_sb = singles.tile([128, CT, N], BF16)

            # ---- emission helpers.  All PE work is emitted via closures so
            # the interleaving below is explicit.
            with (
                tc.tile_pool(name="ps512", bufs=2, space="PSUM") as psp,
                tc.tile_pool(name="st_ps", bufs=2, space="PSUM") as stp,
                tc.tile_pool(name="ov_ps", bufs=2, space="PSUM") as ovp,
            ):
                def qk_proj_chunk(w_sb, b_sb, dst, ct, nch):
                    # one 512-wide chunk: 8 matmuls + bias add
                    ps = psp.tile([128, 512], F32, tag="ps")
                    for dc in range(DC):
                        nc.tensor.matmul(
                            ps[:],
                            w_sb[:, dc, ct * 128:(ct + 1) * 128],
                            xT_sb[:, dc, nch * 512:(nch + 1) * 512],
                            start=(dc == 0),
                            stop=(dc == DC - 1),
                        )
                    nc.vector.tensor_tensor(
                        dst[:, ct, nch * 512:(nch + 1) * 512],
                        ps[:],
                        b_sb[:, ct:ct + 1].to_broadcast((128, 512)),
                        mybir.AluOpType.add,
                    )

                def v_proj_tile(nt):
                    # V natural: v[n, c] = sum_d xT[d, n] wvT[d, c], one n tile
                    ps = psp.tile([128, 512], F32, tag="ps")
                    for dc in range(DC):
                        nc.tensor.matmul(
                            ps[:, :C_LOC],
                            xT_sb[:, dc, nt * 128:(nt + 1) * 128],
                            wvT_sb[:, dc, :],
                            start=(dc == 0),
                            stop=(dc == DC - 1),
                        )
                    # scatter the 4 heads into their [pair, half] slots
                    nc.vector.tensor_copy(
                        v1_sb[:, nt].rearrange("p c h w -> p (c h) w")[:, :, 0:64],
                        ps[:, :C_LOC].rearrange("p (g d) -> p g d", g=4),
                    )

                def y_proj_tile(nt):
                    # output projection for one 128-row tile: ct-outer so each
                    # stationary load feeds both 512-col chunks.
                    pss = [psp.tile([128, 512], F32, tag="ps", name=f"yps{i}")
                           for i in range(D_MODEL // 512)]
                    for ct in range(CT):
                        for cok, ps in enumerate(pss):
                            nc.tensor.matmul(
                                ps[:],
                                outT_sb[:, ct, nt * 128:(nt + 1) * 128],
                                woT_sb[:, ct, cok * 512:(cok + 1) * 512],
                                start=(ct == 0),
                                stop=(ct == CT - 1),
                            )
                    for cok, ps in enumerate(pss):
                        ys = youtp.tile([128, 512], F32, tag="ys")
                        nc.vector.tensor_copy(ys[:], ps[:])
                        nc.sync.dma_start(
                            y.ap()[nt * 128:(nt + 1) * 128,
                                   cok * 512:(cok + 1) * 512],
                            ys[:],
                        )

                def attention_window(qq, pair, filler=None):
                    """One q-window of one head pair.  `filler()` is called
                    once per k-tile to emit interleaved PE work."""
                    q0 = qq * QW
                    ovA = ovp.tile([128, QW], F32, tag="ov")
                    ovB = ovp.tile([128, QW], F32, tag="ov")
                    for kt in range(NT):
                        st = stp.tile([128, 2 * QW], F32)
                        for half, p0 in ((0, 0), (1, 64)):
                            nc.tensor.matmul(
                                st[:, half * QW:(half + 1) * QW],
                                kT_sb[p0:p0 + 64, pair,
                                      kt * 128:(kt + 1) * 128],
                                qT_sb[p0:p0 + 64, pair, q0:q0 + QW],
                                start=True,
                                stop=True,
                            )
                        pt = ptp.tile([128, 2 * QW], BF16)
                        nc.scalar.activation(
                            pt[:], st[:], mybir.ActivationFunctionType.Exp
                        )
                        if filler is not None:
                            filler(kt)
                        # [V | ones] -> out rows 0:64, den rows 64:128
                        for half, ov in ((0, ovA), (1, ovB)):
                            nc.tensor.matmul(
                                ov[:],
                                v1_sb[:, kt, pair, half],
                                pt[:, half * QW:(half + 1) * QW],
                                start=(kt == 0),
                                stop=(kt == NT - 1),
                            )
                    # normalize: rec = 1/den (broadcast across partitions is
                    # already materialized), outT = out * rec.  The custom-DVE
                    # reciprocal misbehaves on PSUM input at partition offset
                    # 64, so bounce den through SBUF first (tensor_copy with
                    # cross partition offsets is fine).
                    for half, ov in ((0, ovA), (1, ovB)):
                        p0 = 64 * half
                        den = recp.tile([64, QW], F32, tag="den")
                        nc.vector.tensor_copy(den[:], ov[64:128, :])
                        rec = recp.tile([64, QW], F32, tag="rec")
                        nc.vector.reciprocal_approx_fast(rec[:], den[:])
                        nc.vector.tensor_mul(
                            outT_sb[p0:p0 + 64, pair, q0:q0 + QW],
                            ov[0:64, :],
                            rec[:],
                        )

                # ---- schedule -------------------------------------------
                # Q/K projections for pair 0, then V for the first tiles so
                # attention can start; V for k-tile kt+2 is emitted inside
                # window (0,0)'s loop.
                for nch in range(NW):
                    qk_proj_chunk(wqT_sb, bq_sb, qT_sb, 0, nch)
                    qk_proj_chunk(wkT_sb, bk_sb, kT_sb, 0, nch)
                v_proj_tile(0)
                v_proj_tile(1)

                def fill_v(kt):
                    if kt + 2 < NT:
                        v_proj_tile(kt + 2)

                attention_window(0, 0, filler=fill_v)

                # pair-0 windows 1..3; spread pair-1 Q/K projection (8
                # chunks) across the first two windows' slack.
                proj1 = []
                for nch in range(NW):
                    proj1.append((wqT_sb, bq_sb, qT_sb, nch))
                    proj1.append((wkT_sb, bk_sb, kT_sb, nch))

                def make_fill_proj(chunks):
                    it = iter(chunks)

                    def fill(kt):
                        if kt % 4 == 1:
                            args = next(it, None)
                            if args is not None:
                                w, b, dst, nch = args
                                qk_proj_chunk(w, b, dst, 1, nch)
                    return fill

                fp = make_fill_proj(proj1)
                attention_window(1, 0, filler=fp)
                attention_window(2, 0, filler=fp)
                attention_window(3, 0, filler=fp)

                # pair-1 windows with trailing y projection interleaved.
                # y window qq is ready once pair-1 window qq is normalized.
                def make_fill_y(qq_ready):
                    chunks = list(range(qq_ready * (QW // 128),
                                        (qq_ready + 1) * (QW // 128)))
                    it = iter(chunks)

                    def fill(kt):
                        if kt % 4 == 0:
                            args = next(it, None)
                            if args is not None:
                                y_proj_tile(args)
                    return fill

                attention_window(0, 1)
                attention_window(1, 1, filler=make_fill_y(0))
                attention_window(2, 1, filler=make_fill_y(1))
                attention_window(3, 1, filler=make_fill_y(2))
                for nt in range(3 * (QW // 128), NW * (QW // 128)):
                    y_proj_tile(nt)

    nc.compile()
    return nc


def kernel(x, Wq, bq, Wk, bk, Wv, bv, Wo, bo):
    x = np.asarray(x, dtype=np.float32)
    Wq = np.asarray(Wq, dtype=np.float32)
    Wk = np.asarray(Wk, dtype=np.float32)
    Wv = np.asarray(Wv, dtype=np.float32)
    Wo = np.asarray(Wo, dtype=np.float32)
    bq = np.asarray(bq, dtype=np.float32)
    bk = np.asarray(bk, dtype=np.float32)
    bv = np.asarray(bv, dtype=np.float32)
    bo = np.asarray(bo, dtype=np.float32)

    if "nc" not in _CACHE:
        _CACHE["nc"] = build_nc()
    nc = _CACHE["nc"]

    s = 2.0 / np.sqrt(8.0)  # fold bipolar *2 and score scale (1/8 split per side)
    in_maps = []
    for core in range(N_CORES):
        b = core // (N_CORES // B)
        g = core % (N_CORES // B)
        ch = slice(g * C_LOC, (g + 1) * C_LOC)
        in_maps.append({
            "xT": np.ascontiguousarray(x[b].T),
            "wqT": np.ascontiguousarray((s * Wq[ch, :]).T),
            "wkT": np.ascontiguousarray((s * Wk[ch, :]).T),
            "wvT": np.ascontiguousarray(Wv[ch, :].T),
            "woT": np.ascontiguousarray(Wo[:, ch].T).astype(ml_dtypes.bfloat16),
            "bq": ((2.0 * bq[ch] - 1.0) / np.sqrt(8.0)).astype(np.float32),
            "bk": ((2.0 * bk[ch] - 1.0) / np.sqrt(8.0)).astype(np.float32),
        })

    _CACHE["in_maps"] = in_maps
    res = run_bass_kernel_spmd(nc, in_maps, core_ids=list(range(N_CORES)))

    g_per_b = N_CORES // B
    const = (Wo @ bv + bo).astype(np.float32)  # bv folded through out-proj
    out = np.empty((B, N, D_MODEL), dtype=np.float32)
    for b in range(B):
        acc = res.results[b * g_per_b]["y"].astype(np.float32).copy()
        for g in range(1, g_per_b):
            acc += res.results[b * g_per_b + g]["y"]
        out[b] = acc + const
    return out
